# revision 24
# baseline (speedup 1.0000x reference)
"""Multi-head attention Trainium2 kernel (B=4, S=2048, E=1024, H=16, D=64).

Sharding: head-parallel x data-parallel. Core c owns heads {2c, 2c+1} for all
4 batches -> 8 (batch, head) jobs per core, no cross-core communication.

All matmuls run in bf16 (fp32r triggers the TRN2 fp32 power throttle that
caps PE utilization at 50%); PSUM accumulation stays fp32.

PE-array tiling: the score matmuls contract over D=64, so two chunks run
concurrently in the two 64-row halves of the 128x128 array (row tiling via
base partitions 0/64). The q/k projections (K=64, M=64) run as a 2x2 tile
grid (4 concurrent matmuls), v projections as row-tiled pairs. For this,
x / W / biases are duplicated across both partition halves.

Per (batch, head) job on device:
  qT = (Wq/8)^T @ xT + bq/8           [128, 2048]   (dup halves; bias via DVE)
  kT = Wk^T @ xT + bk                 [128, 2048]   (dup halves)
  v  = xT^T @ Wv                      per 128-chunk [128, 64]  (no bias)
  scoresT[k, q] = kT_chunk^T @ qT     [128, 512] x2 concurrent (chunk pair)
  attnT = exp(scoresT)                ACT reads PSUM [128, 1024], writes bf16
  outT[65, q] += v_aug_chunk^T @ attnT   accumulated over 16 k-chunks in PSUM;
                                          row 64 = sum_k attnT = softmax denom
                                          (ones col in v_aug via memset)
Host side: shard/unshard reshapes, out = num/denom + bv (v-bias folds out
exactly because sum_k attn = denom), bias/scale folding.
"""

import numpy as np
import ml_dtypes

import concourse.bass as bass
import concourse.mybir as mybir
import concourse.tile as tile
from concourse.bass_utils import run_bass_kernel_spmd

F32 = mybir.dt.float32
BF16 = mybir.dt.bfloat16

B, S, E, H = 4, 2048, 1024, 16
D = E // H            # 64
NCORES = 8
HPC = H // NCORES     # heads per core = 2
PAIRS = B * HPC       # jobs per core = 8
NQ = 512              # q-group width
NG = S // NQ          # 4 q groups
KC = S // 128         # 16 k chunks of 128
KP = KC // 2          # 8 chunk pairs
VW = 2 * (D + 1)      # 130: v pair block [cA, onesA, cB, onesB]


def _patched_drain_and_barrier(self, tick_clock, wait_clock):
    # This walrus build rejects >1 sync-wait on a Drain (CTRL) instruction.
    # Collect the TileContext-exit waits on individual NOPs instead.
    nc = self.nc
    collector = nc.sync.nop(nofuse=True)
    wait_clock.add_sem_waits(
        collector.ins, tile.ScopedClock({None: tick_clock.global_clock})
    )
    si = collector.ins.sync_info
    if si is not None and len(si.on_wait) > 1:
        waits = list(si.on_wait)
        collector.ins.sync_info = mybir.SyncInfo(
            on_wait=[waits[0]], on_update=list(si.on_update)
        )
        for w in waits[1:]:
            n2 = nc.sync.nop(nofuse=True)
            n2.ins.sync_info = mybir.SyncInfo(on_wait=[w], on_update=[])
    nc.sync.drain()
    popped = nc._tile_sem_poison_stack.pop()
    assert popped is self._sem_poison


tile.TileContext._drain_and_barrier = _patched_drain_and_barrier

_MAX_WAITS = 1


def _split_excess_waits(nc):
    """This walrus build allows at most one sync-wait per instruction; hoist
    extra waits onto NOPs inserted immediately before, on the same engine."""
    n = 0
    for f in nc.m.functions:
        for bb in f.blocks:
            new_insts = []
            for inst in bb.instructions:
                si = inst.sync_info
                if si is not None and len(si.on_wait) > _MAX_WAITS:
                    waits = list(si.on_wait)
                    for w in waits[:-_MAX_WAITS]:
                        nop = mybir.InstNoOp(
                            name=f"waitnop-{n}",
                            engine=inst.engine,
                            ins=[],
                            outs=[],
                            sync_info=mybir.SyncInfo(on_wait=[w], on_update=[]),
                            bass_nofuse=True,
                        )
                        n += 1
                        new_insts.append(nop)
                    inst.sync_info = mybir.SyncInfo(
                        on_wait=waits[-_MAX_WAITS:],
                        on_update=list(si.on_update),
                    )
                new_insts.append(inst)
            bb.instructions = new_insts


_NC_CACHE = {}


def build_nc():
    if "nc" in _NC_CACHE:
        return _NC_CACHE["nc"]
    nc = bass.Bass()
    xt = nc.dram_tensor("xt", [PAIRS, 128, S], BF16, kind="ExternalInput")
    wall = nc.dram_tensor("wall", [128, HPC * 3 * D], BF16,
                          kind="ExternalInput")
    ball = nc.dram_tensor("ball", [128, HPC * 2], F32, kind="ExternalInput")
    out = nc.dram_tensor("out", [PAIRS, D + 1, S], F32, kind="ExternalOutput")

    with tile.TileContext(nc) as tc:
        with (
            tc.tile_pool(name="sb", bufs=2) as sb,
            tc.tile_pool(name="at", bufs=4) as atp,
            tc.tile_pool(name="wp", bufs=1) as wp,
            tc.tile_pool(name="sp", bufs=3, space="PSUM") as sp,
            tc.tile_pool(name="op", bufs=2, space="PSUM") as op,
        ):
            # weights/biases resident for the whole kernel: 2 packed DMAs
            wb = wp.tile([128, HPC * 3 * D], BF16, tag="wb")
            nc.sync.dma_start(wb[:], wall[:, :])
            bb = wp.tile([128, HPC * 2], F32, tag="bb")
            nc.sync.dma_start(bb[:], ball[:, :])
            w_t = {}
            for jj in range(HPC):
                o = jj * 3 * D
                w_t["w2", jj] = wb[:, o: o + 2 * D]      # [Wq|Wq ; Wk|Wk]
                w_t["wv2", jj] = wb[:, o + 2 * D: o + 3 * D]
                w_t["bq2", jj] = bb[:, 2 * jj: 2 * jj + 1]
                w_t["bk2", jj] = bb[:, 2 * jj + 1: 2 * jj + 2]

            def load_pair(p):
                t = sb.tile([128, S], BF16, tag="xt")
                for h in range(2):
                    sl = bass.ts(h, S // 2)
                    nc.sync.dma_start(t[:, sl], xt[p][:, sl])
                return t

            def proj_qk(xt_t, jj, qt2, kt2, g):
                # stationary [Wq|Wq] (64x128) -> out [q;q] across all 128
                # partitions in one standard matmul; same for [Wk|Wk] using
                # the x copy at rows 64-127 (row groups 0/64 overlap)
                sl = bass.ts(g, NQ)
                ps = sp.tile([128, 2 * NQ], F32, tag="s")
                wj = w_t["w2", jj]
                nc.tensor.matmul(ps[:, 0:NQ], wj[0:D], xt_t[0:D, sl],
                                 start=True, stop=True)
                nc.tensor.matmul(ps[:, NQ:], wj[D:128], xt_t[D:128, sl],
                                 start=True, stop=True)
                nc.vector.tensor_scalar_add(qt2[:, sl], ps[:, 0:NQ],
                                            w_t["bq2", jj][:])
                nc.vector.tensor_scalar_add(kt2[:, sl], ps[:, NQ:],
                                            w_t["bk2", jj][:])

            def proj_v_pair(xt_t, jj, v_t, i):
                # serial chunk pair; outputs in different PSUM banks
                psv = sp.tile([128, 2 * NQ], F32, tag="s")
                wvj = w_t["wv2", jj]
                nc.tensor.matmul(psv[:, 0:D], xt_t[0:D, bass.ts(2 * i, 128)],
                                 wvj[0:D], start=True, stop=True)
                nc.tensor.matmul(psv[:, NQ:NQ + D],
                                 xt_t[0:D, bass.ts(2 * i + 1, 128)],
                                 wvj[0:D], start=True, stop=True)
                vb = i * VW
                nc.vector.tensor_copy(v_t[:, vb:vb + D], psv[:, 0:D])
                nc.vector.tensor_copy(v_t[:, vb + D + 1:vb + 2 * D + 1],
                                      psv[:, NQ:NQ + D])

            def new_job_tiles():
                qt2 = sb.tile([128, S], BF16, tag="qt")
                kt2 = sb.tile([128, S], BF16, tag="kt")
                v_t = sb.tile([128, KP * VW], BF16, tag="v")
                nc.vector.memset(v_t[:], 1.0)
                return qt2, kt2, v_t

            cur = load_pair(0)
            cur_tiles = new_job_tiles()
            proj_qk(cur, 0, cur_tiles[0], cur_tiles[1], 0)
            proj_qk(cur, 0, cur_tiles[0], cur_tiles[1], 1)
            proj_v_pair(cur, 0, cur_tiles[2], 0)
            proj_v_pair(cur, 0, cur_tiles[2], 1)
            nxt = nxt_tiles = None
            for p in range(PAIRS):
                j = p % HPC
                xt_t = cur
                qt2, kt2, v_t = cur_tiles

                if p + 1 < PAIRS:
                    nxt = load_pair(p + 1)
                    nxt_tiles = new_job_tiles()

                for g in range(NG):
                    qsl = bass.ts(g, NQ)
                    out_ps = op.tile([D + 1, NQ], F32, tag="out")
                    pend = []
                    for i in range(KP):
                        sps = sp.tile([128, 2 * NQ], F32, tag="s")
                        nc.tensor.matmul(sps[:, 0:NQ],
                                         kt2[0:D, bass.ts(2 * i, 128)],
                                         qt2[0:D, qsl],
                                         start=True, stop=True)
                        nc.tensor.matmul(sps[:, NQ:],
                                         kt2[D:128, bass.ts(2 * i + 1, 128)],
                                         qt2[D:128, qsl],
                                         start=True, stop=True)
                        at = atp.tile([128, 2 * NQ], BF16, tag="attn")
                        nc.scalar.activation(at[:], sps[:],
                                             mybir.ActivationFunctionType.Exp)
                        if g == 0:
                            if p == 0:
                                # spread: <=1 sp-ring alloc per iteration so
                                # early exps aren't starved of sps buffers
                                if i == 0:
                                    proj_v_pair(xt_t, j, v_t, 2)
                                elif i == 1:
                                    proj_qk(xt_t, j, qt2, kt2, 2)
                                    proj_v_pair(xt_t, j, v_t, 3)
                                elif i == 2:
                                    proj_qk(xt_t, j, qt2, kt2, 3)
                                    proj_v_pair(xt_t, j, v_t, 4)
                                elif i < 6:
                                    proj_v_pair(xt_t, j, v_t, i + 2)
                            else:
                                if i < KP - 4:
                                    proj_v_pair(xt_t, j, v_t, i + 4)
                        elif g == NG - 2 and p + 1 < PAIRS:
                            jn = (p + 1) % HPC
                            if i == 0:
                                proj_qk(nxt, jn, nxt_tiles[0], nxt_tiles[1], 0)
                            elif i == 1:
                                proj_v_pair(nxt, jn, nxt_tiles[2], 0)
                            elif i == 2:
                                proj_qk(nxt, jn, nxt_tiles[0], nxt_tiles[1], 1)
                            elif i == 3:
                                proj_qk(nxt, jn, nxt_tiles[0], nxt_tiles[1], 2)
                        elif g == NG - 1 and p + 1 < PAIRS:
                            jn = (p + 1) % HPC
                            if i < 3:
                                proj_v_pair(nxt, jn, nxt_tiles[2], i + 1)
                            elif i == 3:
                                proj_qk(nxt, jn, nxt_tiles[0], nxt_tiles[1], 3)
                        if len(pend) == 2:
                            pat, pi = pend.pop(0)
                            vb = pi * VW
                            nc.tensor.matmul(out_ps[:], v_t[:, vb:vb + D + 1],
                                             pat[:, 0:NQ],
                                             start=(pi == 0), stop=False)
                            nc.tensor.matmul(out_ps[:],
                                             v_t[:, vb + D + 1:vb + VW],
                                             pat[:, NQ:],
                                             start=False, stop=False)
                        pend.append((at, i))
                    for fi, (pat, pi) in enumerate(pend):
                        vb = pi * VW
                        last = fi == len(pend) - 1
                        nc.tensor.matmul(out_ps[:], v_t[:, vb:vb + D + 1],
                                         pat[:, 0:NQ],
                                         start=(pi == 0), stop=False)
                        nc.tensor.matmul(out_ps[:], v_t[:, vb + D + 1:vb + VW],
                                         pat[:, NQ:],
                                         start=False, stop=last)

                    # un-normalized [numerator; denominator] to HBM via an
                    # SBUF staging copy (DMA cannot read PSUM); the host
                    # divides rows 0:64 by row 64 during unshard
                    o_t = sb.tile([D + 1, NQ], F32, tag="o")
                    nc.vector.tensor_copy(o_t[:], out_ps[:])
                    nc.sync.dma_start(out[p, :, qsl], o_t[:])
                cur, cur_tiles = nxt, nxt_tiles

    _split_excess_waits(nc)
    _NC_CACHE["nc"] = nc
    return nc


def _prep_inputs(sequences, Wq, bq, Wk, bk, Wv, bv):
    s = 1.0 / np.sqrt(D)
    x = np.ascontiguousarray(np.asarray(sequences, dtype=np.float32))
    xh = x.reshape(B, S, H, D).transpose(2, 0, 3, 1)      # [H, B, D, S]
    x2 = np.concatenate([xh, xh], axis=2)                 # [H, B, 128, S]
    x2 = x2.astype(ml_dtypes.bfloat16)

    Wq = np.asarray(Wq, np.float32) * s
    Wk = np.asarray(Wk, np.float32)
    Wv = np.asarray(Wv, np.float32)
    bqs = np.asarray(bq, np.float32) * s
    bks = np.asarray(bk, np.float32)

    w2 = np.concatenate([Wq, Wk], axis=1).astype(ml_dtypes.bfloat16)
    wv2 = np.concatenate([Wv, Wv], axis=1).astype(ml_dtypes.bfloat16)
    bq2 = np.concatenate([bqs, bqs], axis=1).astype(np.float32)   # [H,128]
    bk2 = np.concatenate([bks, bks], axis=1).astype(np.float32)

    in_maps = []
    for c in range(NCORES):
        xt_core = np.ascontiguousarray(np.stack(
            [x2[HPC * c + jj, b] for b in range(B) for jj in range(HPC)]))
        # wall[:, j*128 : j*128+64] = w2_j ; [.. +64 : +128] = wv2_j
        wall = np.empty((128, HPC * 3 * D), ml_dtypes.bfloat16)
        ball = np.empty((128, HPC * 2), np.float32)
        for jj in range(HPC):
            h = HPC * c + jj
            o = jj * 3 * D
            wall[:, o: o + D] = w2[h]          # rows 0-63: Wq, 64-127: Wk
            wall[:, o + D: o + 2 * D] = w2[h]  # duplicated along free dim
            wall[:, o + 2 * D: o + 3 * D] = wv2[h]
            ball[:, 2 * jj] = bq2[h]
            ball[:, 2 * jj + 1] = bk2[h]
        in_maps.append({
            "xt": xt_core,
            "wall": np.ascontiguousarray(wall),
            "ball": np.ascontiguousarray(ball),
        })
    return in_maps


def _assemble(results, bv):
    bv = np.asarray(bv, np.float32)
    out = np.empty((B, S, E), np.float32)
    for c in range(NCORES):
        r = results[c]["out"]                              # [8, 65, 2048]
        for b in range(B):
            for jj in range(HPC):
                h = HPC * c + jj
                rp = r[HPC * b + jj]
                out[b, :, h * D:(h + 1) * D] = (
                    (rp[:D] / rp[D:D + 1]).T + bv[h][None, :])
    return out


def run(trace=False, **inputs):
    nc = build_nc()
    in_maps = _prep_inputs(**inputs)
    res = run_bass_kernel_spmd(nc, in_maps, list(range(NCORES)), trace=trace)
    return _assemble(res.results, inputs["bv"]), res


def kernel(**inputs):
    out, _ = run(trace=False, **inputs)
    return out
